# revision 12
# baseline (speedup 1.0000x reference)
"""Trainium2 Bass kernel for a dense transformer encoder layer.

Shapes: B=4, S=2048, D=1024, H=16 heads x DH=64.
Sharding: sequence-parallel over 8 cores — core c handles batch c//2,
query rows (c%2)*1024..+1024; K/V computed for the full batch sequence
on both cores of a pair (duplicated, no collectives).

Layout strategy: all on-chip activations are kept transposed
[features -> partitions (tiles of 128), tokens -> free dim] so that every
matmul contraction lands on the partition axis without any on-chip
transposes.  The host pre-transposes x and post-transposes the output.

Precision: projections run fp32r (full PE rate at N=512); attention
tensors (Q^T, K^T, V, exp-scores) are bf16 with fp32 PSUM accumulation.
Softmax skips max-subtraction (scores ~ N(0,1) after the 1/sqrt(dh)
scale, folded into Exp's affine); the softmax denominator is obtained by
appending a ones-column to V so that attn @ [V|1] yields both the
context and the sum of exponentials in one accumulation.

SBUF is a two-sided stack allocator: long-lived tensors go on the left
stack, phase-transient ones on the right, freed in LIFO order per side.
"""

import numpy as np

import concourse.bass as bass
import concourse.mybir as mybir
import concourse.tile as tile
from concourse import bacc
from concourse.bass_utils import run_bass_kernel_spmd

F32 = mybir.dt.float32
F32R = mybir.dt.float32r
BF16 = mybir.dt.bfloat16
AF = mybir.ActivationFunctionType

B, S, D, H = 4, 2048, 1024, 16
DH = D // H          # 64
NT = D // 128        # 8 feature tiles
KTT = S // 128       # 16 key-token tiles
MQ = S // 2          # 1024 local query rows per core
QC = MQ // 512       # 2 query chunks of 512
EPS = 1e-6


def r(ap):
    """bitcast an fp32 AP to fp32r for full-rate PE matmuls"""
    return ap.bitcast(F32R)


def build_nc():
    nc = bacc.Bacc("TRN2", target_bir_lowering=False, debug=False, num_devices=8)

    xkvT = nc.declare_dram_parameter("xkvT", [D, S], F32, isOutput=False)
    xqT = nc.declare_dram_parameter("xqT", [D, MQ], F32, isOutput=False)
    w_par = {}
    b_par = {}
    for nm in ("wq", "wk", "wv", "wo", "wf"):
        w_par[nm] = nc.declare_dram_parameter(nm, [D, D], F32, isOutput=False)
    for nm in ("bq", "bk", "bv", "bo", "bf", "g1", "b1", "g2", "b2"):
        b_par[nm] = nc.declare_dram_parameter(nm, [D], F32, isOutput=False)
    outT = nc.declare_dram_parameter("outT", [D, MQ], F32, isOutput=True)

    with tile.TileContext(nc) as tc:
        keepalive = build_body(nc, tc, xkvT, xqT, w_par, b_par, outT)

    nc.compile()
    del keepalive
    return nc


def load_wcol(nc, pool, w, nt):
    """weight column block w[:, nt*128:(nt+1)*128] -> SBUF [128, NT, 128]"""
    t = pool.tile([128, NT, 128], F32R, name=f"wcol_{w.name}_{nt}", tag="wcol")
    src = w[:, nt * 128:(nt + 1) * 128].rearrange("(t p) n -> p t n", p=128)
    nc.sync.dma_start(out=t, in_=src.bitcast(F32R))
    return t



def bcast_row(nc, out_ap, row_ap, nparts):
    """replicate a [1, N] SBUF row across nparts partitions via DMA"""
    inner = list(row_ap.ap[-1])
    src = bass.AP(tensor=row_ap.tensor, offset=row_ap.offset,
                  ap=[[1, 1], [0, nparts], inner])
    nc.sync.dma_start(out=out_ap, in_=src)

def build_body(nc, tc, xkvT, xqT, w_par, b_par, outT):
    f32 = F32

    # ---------------- constants / biases (whole-kernel, left stack) ----------
    final_frees = []  # popped in reverse at the end (left-stack LIFO)
    bias_sb = {}
    for nm in ("bq", "bk", "bo", "bf", "g1", "b1", "g2", "b2"):
        t, fr = tc.tile([128, NT], f32, name=f"bias_{nm}", side="left")
        nc.sync.dma_start(out=t, in_=b_par[nm][:].rearrange("(t p) -> p t", p=128))
        bias_sb[nm] = t
        final_frees.append(fr)
    # bv broadcast across partitions [128, D] (free-dim bias for natural-layout V)
    bvb, bvb_fr = tc.tile([128, D], f32, name="bvb", side="left")
    final_frees.append(bvb_fr)
    bv_ap = b_par["bv"][:]
    bv_bc = bass.AP(tensor=bv_ap.tensor, offset=bv_ap.offset, ap=[[0, 128], [1, D]])
    nc.sync.dma_start(out=bvb, in_=bv_bc)
    ones_sb, ones_fr = tc.tile([128, 1], F32R, name="ones_sb", side="left")
    final_frees.append(ones_fr)
    ones_f32, ones_f32_fr = tc.tile([128, 1], f32, name="ones_f32", side="left")
    final_frees.append(ones_f32_fr)
    nc.vector.memset(ones_f32, 1.0)
    nc.scalar.copy(out=ones_sb, in_=ones_f32)
    eps_sb, eps_fr = tc.tile([1, 1], f32, name="eps_sb", side="left")
    final_frees.append(eps_fr)
    nc.vector.memset(eps_sb, EPS)

    # ---------------- persistent attention tensors (left stack) --------------
    QT, QT_free = tc.tile([128, NT, MQ], BF16, name="QT", side="left")
    KT, KT_free = tc.tile([128, NT, S], BF16, name="KT", side="left")
    VA, VA_free = tc.tile([128, KTT, H, DH + 1], BF16, name="VA", side="left")

    xkv_sb, xkv_free = tc.tile([128, NT, S], F32R, name="xkv_sb", side="right")
    nc.sync.dma_start(
        out=xkv_sb, in_=xkvT[:, :].rearrange("(t p) m -> p t m", p=128).bitcast(F32R))

    # ======================= P1: Q, K, V projections =========================
    with tc.tile_pool(name="p1ps", bufs=4, space="PSUM") as p1ps:
        # ---- Q^T = (wq^T x^T + bq) : [D, MQ], bf16 out
        xq_sb, xq_free = tc.tile([128, NT, MQ], F32R, name="xq_sb", side="right")
        nc.sync.dma_start(
            out=xq_sb,
            in_=xqT[:, :].rearrange("(t p) m -> p t m", p=128).bitcast(F32R))
        with tc.tile_pool(name="wpool", bufs=2, side="left") as wpool:
            for nt in range(NT):
                wc = load_wcol(nc, wpool, w_par["wq"], nt)
                for qc in range(QC):
                    ps = p1ps.tile([128, 512], f32, name="ps_q", tag="p1")
                    for kt in range(NT):
                        nc.tensor.matmul(
                            ps, wc[:, kt, :],
                            xq_sb[:, kt, qc * 512:(qc + 1) * 512],
                            start=(kt == 0), stop=(kt == NT - 1))
                    nc.vector.tensor_scalar(
                        out=QT[:, nt, qc * 512:(qc + 1) * 512], in0=ps,
                        scalar1=bias_sb["bq"][:, nt:nt + 1], scalar2=None,
                        op0=mybir.AluOpType.add)

            # ---- K^T over full sequence: [D, S]
            for nt in range(NT):
                wc = load_wcol(nc, wpool, w_par["wk"], nt)
                for mc in range(S // 512):
                    ps = p1ps.tile([128, 512], f32, name="ps_k", tag="p1")
                    for kt in range(NT):
                        nc.tensor.matmul(
                            ps, wc[:, kt, :],
                            xkv_sb[:, kt, mc * 512:(mc + 1) * 512],
                            start=(kt == 0), stop=(kt == NT - 1))
                    nc.vector.tensor_scalar(
                        out=KT[:, nt, mc * 512:(mc + 1) * 512], in0=ps,
                        scalar1=bias_sb["bk"][:, nt:nt + 1], scalar2=None,
                        op0=mybir.AluOpType.add)
        xq_free()

        # ---- V natural layout + bias + ones column -> VA [tok, head, 65]
        with tc.tile_pool(name="wvpool", bufs=2, side="left") as wvpool:
            for dc in range(2):  # d-range halves of 512
                wv_rows = wvpool.tile([128, NT, 512], F32R,
                                      name=f"wv_rows_{dc}", tag="wvrows")
                nc.sync.dma_start(
                    out=wv_rows,
                    in_=w_par["wv"][:, dc * 512:(dc + 1) * 512].rearrange(
                        "(t p) n -> p t n", p=128).bitcast(F32R))
                for tt in range(KTT):
                    ps = p1ps.tile([128, 512], f32, name="ps_v", tag="p1")
                    for kt in range(NT):
                        nc.tensor.matmul(
                            ps, xkv_sb[:, kt, tt * 128:(tt + 1) * 128],
                            wv_rows[:, kt, :],
                            start=(kt == 0), stop=(kt == NT - 1))
                    nc.vector.tensor_add(
                        out=VA[:, tt, dc * 8:(dc + 1) * 8, 0:DH],
                        in0=ps.rearrange("p (h d) -> p h d", h=8),
                        in1=bvb[:, dc * 512:(dc + 1) * 512].rearrange(
                            "p (h d) -> p h d", h=8))
            for tt in range(KTT):
                nc.vector.memset(VA[:, tt, :, DH:DH + 1], 1.0)

    xkv_free()

    # ======================= P2: attention ===================================
    ctxT, ctxT_free = tc.tile([128, NT, MQ], F32R, name="ctxT", side="right")

    with tc.tile_pool(name="exppool", bufs=4, side="left") as exppool, \
         tc.tile_pool(name="sepool", bufs=4, side="left") as sepool, \
         tc.tile_pool(name="bcpool", bufs=2, side="left") as bcpool, \
         tc.tile_pool(name="sps", bufs=3, space="PSUM") as sps, \
         tc.tile_pool(name="cps", bufs=2, space="PSUM") as cps:

        for hp in range(H // 2):
            h0, h1 = 2 * hp, 2 * hp + 1
            se0 = sepool.tile([1, MQ], f32, name="se0", tag="se")
            se1 = sepool.tile([1, MQ], f32, name="se1", tag="se")
            for qc in range(QC):
                qs = slice(qc * 512, (qc + 1) * 512)
                pc0 = cps.tile([128, 512], f32, name="pc0", tag="pc")
                pc1 = cps.tile([128, 512], f32, name="pc1", tag="pc")
                for half in range(2):
                    e0 = exppool.tile([128, 8, 512], BF16, name="e0", tag="exp")
                    e1 = exppool.tile([128, 8, 512], BF16, name="e1", tag="exp")
                    # scores^T for 8 k-token tiles, 2 heads row-packed
                    for g in range(4):
                        ps0 = sps.tile([128, 2, 512], f32, name="ps0", tag="sc")
                        ps1 = sps.tile([128, 2, 512], f32, name="ps1", tag="sc")
                        for j in range(2):
                            kt = half * 8 + g * 2 + j
                            ks = slice(kt * 128, (kt + 1) * 128)
                            nc.tensor.matmul(
                                ps0[:, j, :], KT[0:64, hp, ks], QT[0:64, hp, qs],
                                start=True, stop=True, tile_position=(0, 0))
                            nc.tensor.matmul(
                                ps1[:, j, :], KT[64:128, hp, ks], QT[64:128, hp, qs],
                                start=True, stop=True, tile_position=(64, 0))
                        nc.scalar.activation(
                            out=e0[:, g * 2:(g + 1) * 2, :], in_=ps0,
                            func=AF.Exp, scale=1.0 / np.sqrt(DH))
                        nc.scalar.activation(
                            out=e1[:, g * 2:(g + 1) * 2, :], in_=ps1,
                            func=AF.Exp, scale=1.0 / np.sqrt(DH))
                    # ctx accumulation for this half
                    for j in range(8):
                        kt = half * 8 + j
                        nc.tensor.matmul(
                            pc0[0:DH + 1, :], VA[:, kt, h0, :], e0[:, j, :],
                            start=(kt == 0), stop=(kt == KTT - 1))
                        nc.tensor.matmul(
                            pc1[0:DH + 1, :], VA[:, kt, h1, :], e1[:, j, :],
                            start=(kt == 0), stop=(kt == KTT - 1))
                nc.vector.reciprocal(out=se0[0:1, qs], in_=pc0[DH:DH + 1, :])
                nc.vector.reciprocal(out=se1[0:1, qs], in_=pc1[DH:DH + 1, :])
                nc.vector.tensor_copy(out=ctxT[0:64, hp, qs], in_=pc0[0:DH, :])
                nc.vector.tensor_copy(out=ctxT[64:128, hp, qs], in_=pc1[0:DH, :])
            # normalize ctx by 1/sum_exp (per head, broadcast over DH partitions)
            bc = bcpool.tile([128, MQ], f32, name="bc", tag="bc")
            bcast_row(nc, bc[0:64, :], se0[0:1, :], 64)
            bcast_row(nc, bc[64:128, :], se1[0:1, :], 64)
            nc.vector.tensor_mul(out=ctxT[:, hp, :], in0=ctxT[:, hp, :], in1=bc)

    VA_free()
    KT_free()
    QT_free()

    # ======================= P3: output projection + residual ================
    y1, y1_free = tc.tile([128, NT, MQ], F32R, name="y1", side="left")
    xq2, xq2_free = tc.tile([128, NT, MQ], f32, name="xq2", side="right")
    nc.sync.dma_start(out=xq2, in_=xqT[:, :].rearrange("(t p) m -> p t m", p=128))

    with tc.tile_pool(name="wpool3", bufs=2, side="left") as wpool3, \
         tc.tile_pool(name="p3ps", bufs=4, space="PSUM") as p3ps:
        for nt in range(NT):
            wc = load_wcol(nc, wpool3, w_par["wo"], nt)
            for qc in range(QC):
                qs = slice(qc * 512, (qc + 1) * 512)
                ps = p3ps.tile([128, 512], f32, name="ps_o", tag="p3")
                for kt in range(NT):
                    nc.tensor.matmul(
                        ps, wc[:, kt, :], ctxT[:, kt, qs],
                        start=(kt == 0), stop=(kt == NT - 1))
                nc.vector.tensor_scalar(
                    out=y1[:, nt, qs], in0=ps,
                    scalar1=bias_sb["bo"][:, nt:nt + 1], scalar2=None,
                    op0=mybir.AluOpType.add)
                nc.vector.tensor_add(out=y1[:, nt, qs], in0=y1[:, nt, qs],
                                     in1=xq2[:, nt, qs])
    xq2_free()
    ctxT_free()

    # ======================= P4: LayerNorm 1 =================================
    n1, n1_free = tc.tile([128, NT, MQ], F32R, name="n1", side="right")
    layernorm(nc, tc, src=y1, dst=n1, g_sb=bias_sb["g1"], b_sb=bias_sb["b1"],
              ones_sb=ones_sb, eps_sb=eps_sb, label="ln1")
    y1_free()

    # ======================= P5: FFN ==========================================
    fnn, fnn_free = tc.tile([128, NT, MQ], F32R, name="fnn", side="left")
    with tc.tile_pool(name="wpool5", bufs=2, side="left") as wpool5, \
         tc.tile_pool(name="p5ps", bufs=4, space="PSUM") as p5ps:
        for nt in range(NT):
            wc = load_wcol(nc, wpool5, w_par["wf"], nt)
            for qc in range(QC):
                qs = slice(qc * 512, (qc + 1) * 512)
                ps = p5ps.tile([128, 512], f32, name="ps_f", tag="p5")
                for kt in range(NT):
                    nc.tensor.matmul(
                        ps, wc[:, kt, :], n1[:, kt, qs],
                        start=(kt == 0), stop=(kt == NT - 1))
                # fnn = relu(psum + bf)
                nc.scalar.activation(
                    out=fnn[:, nt, qs], in_=ps, func=AF.Relu,
                    bias=bias_sb["bf"][:, nt:nt + 1], scale=1.0)

    # y2 = fnn + n1 (overwrite fnn)
    for nt in range(NT):
        nc.vector.tensor_add(out=fnn[:, nt, :], in0=fnn[:, nt, :], in1=n1[:, nt, :])
    n1_free()

    # ======================= P6: LayerNorm 2 -> output =======================
    with tc.tile_pool(name="outpool", bufs=2, side="left") as outpool:
        ab = ln_stats(nc, tc, src=fnn, ones_sb=ones_sb, eps_sb=eps_sb, label="ln2")
        for nt in range(NT):
            n2t = outpool.tile([128, MQ], f32, name="n2t", tag="n2")
            apply_ln(nc, n2t, fnn[:, nt, :], ab,
                     bias_sb["g2"][:, nt:nt + 1], bias_sb["b2"][:, nt:nt + 1])
            nc.sync.dma_start(out=outT[nt * 128:(nt + 1) * 128, :], in_=n2t)
        for fr in ab[2]:
            fr()

    # pop remaining left-stack singles in LIFO order
    fnn_free()
    for fr in reversed(final_frees):
        fr()
    return []


def ln_stats(nc, tc, src, ones_sb, eps_sb, label):
    """column sums of src and src^2 over all D partitions via ones-matmuls;
    returns (Abc, Bbc, frees): broadcast tiles with A=rstd, B=-mean*rstd."""
    A, A_free = tc.tile([1, MQ], F32, name=f"A_{label}", side="right")
    Bt, B_free = tc.tile([1, MQ], F32, name=f"B_{label}", side="right")
    m_sb, m_free = tc.tile([1, MQ], F32, name=f"m_{label}", side="right")
    with tc.tile_pool(name=f"sq_{label}", bufs=2, side="left") as sqpool, \
         tc.tile_pool(name=f"lnps_{label}", bufs=2, space="PSUM") as lnps:
        for qc in range(QC):
            qs = slice(qc * 512, (qc + 1) * 512)
            ps_sum = lnps.tile([128, 512], F32, name="ps_sum", tag="lnsum")
            ps_sq = lnps.tile([128, 512], F32, name="ps_sq", tag="lnsum")
            for nt in range(NT):
                sq = sqpool.tile([128, 512], F32R, name="sq", tag="sq")
                nc.vector.tensor_mul(out=sq, in0=src[:, nt, qs], in1=src[:, nt, qs])
                nc.tensor.matmul(ps_sum[0:1, :], ones_sb[:, :], src[:, nt, qs],
                                 start=(nt == 0), stop=(nt == NT - 1))
                nc.tensor.matmul(ps_sq[0:1, :], ones_sb[:, :], sq[:, :],
                                 start=(nt == 0), stop=(nt == NT - 1))
            # mean, var, A = 1/sqrt(var+eps), B = -mean*A  (on [1, 512])
            nc.scalar.mul(out=m_sb[0:1, qs], in_=ps_sum[0:1, :], mul=1.0 / D)
            nc.scalar.mul(out=A[0:1, qs], in_=ps_sq[0:1, :], mul=1.0 / D)
            nc.vector.tensor_mul(out=Bt[0:1, qs], in0=m_sb[0:1, qs], in1=m_sb[0:1, qs])
            nc.vector.tensor_sub(out=A[0:1, qs], in0=A[0:1, qs], in1=Bt[0:1, qs])
            nc.scalar.activation(out=A[0:1, qs], in_=A[0:1, qs], func=AF.Sqrt,
                                 bias=eps_sb[0:1, 0:1], scale=1.0)
            nc.vector.reciprocal(out=A[0:1, qs], in_=A[0:1, qs])
            nc.vector.tensor_mul(out=Bt[0:1, qs], in0=m_sb[0:1, qs], in1=A[0:1, qs])
            nc.scalar.mul(out=Bt[0:1, qs], in_=Bt[0:1, qs], mul=-1.0)
    m_free()
    Abc, Abc_free = tc.tile([128, MQ], F32, name=f"Abc_{label}", side="right")
    Bbc, Bbc_free = tc.tile([128, MQ], F32, name=f"Bbc_{label}", side="right")
    bcast_row(nc, Abc[:, :], A[0:1, :], 128)
    bcast_row(nc, Bbc[:, :], Bt[0:1, :], 128)
    return (Abc, Bbc, (Bbc_free, Abc_free, B_free, A_free))


def apply_ln(nc, out_ap, y_ap, ab, g_col, b_col):
    Abc, Bbc, _ = ab
    nc.vector.tensor_mul(out=out_ap, in0=y_ap, in1=Abc)
    nc.vector.tensor_add(out=out_ap, in0=out_ap, in1=Bbc)
    nc.vector.tensor_scalar(
        out=out_ap, in0=out_ap, scalar1=g_col, scalar2=b_col,
        op0=mybir.AluOpType.mult, op1=mybir.AluOpType.add)


def layernorm(nc, tc, src, dst, g_sb, b_sb, ones_sb, eps_sb, label):
    ab = ln_stats(nc, tc, src=src, ones_sb=ones_sb, eps_sb=eps_sb, label=label)
    for nt in range(NT):
        apply_ln(nc, dst[:, nt, :], src[:, nt, :], ab,
                 g_sb[:, nt:nt + 1], b_sb[:, nt:nt + 1])
    for fr in ab[2]:
        fr()


_NC_CACHE = {}


def _get_nc():
    if "nc" not in _NC_CACHE:
        _NC_CACHE["nc"] = build_nc()
    return _NC_CACHE["nc"]


def kernel(x, wq, bq, wk, bk, wv, bv, wo, bo, wf, bf, g1, b1, g2, b2,
           _trace=False):
    nc = _get_nc()
    x = np.ascontiguousarray(np.asarray(x, dtype=np.float32))
    weights = {}
    for nm, v in (("wq", wq), ("wk", wk), ("wv", wv), ("wo", wo), ("wf", wf),
                  ("bq", bq), ("bk", bk), ("bv", bv), ("bo", bo), ("bf", bf),
                  ("g1", g1), ("b1", b1), ("g2", g2), ("b2", b2)):
        weights[nm] = np.ascontiguousarray(np.asarray(v, dtype=np.float32))
    in_maps = []
    for c in range(8):
        b, half = divmod(c, 2)
        xT = np.ascontiguousarray(x[b].T)                              # [D, S]
        xqTc = np.ascontiguousarray(x[b, half * MQ:(half + 1) * MQ].T)  # [D, MQ]
        in_maps.append({"xkvT": xT, "xqT": xqTc, **weights})
    res = run_bass_kernel_spmd(nc, in_maps, core_ids=list(range(8)), trace=_trace)
    out = np.empty((B, S, D), dtype=np.float32)
    for c in range(8):
        b, half = divmod(c, 2)
        out[b, half * MQ:(half + 1) * MQ, :] = res.results[c]["outT"].T
    if _trace:
        return out, res
    return out
